# revision 1
# baseline (speedup 1.0000x reference)
"""CLIPAttention (B=8, S=1024, D=1024, H=16) Trainium2 Bass kernel.

Strategy: data-parallel over batch — one batch element per NeuronCore (8 cores).
Per core, the whole attention layer runs on-chip:

  - Host pre-transposes/casts weights + hidden states to bf16 (fp32 PSUM accum).
  - qT/kT projections produce Q^T/K^T in [d, s] layout (d on partitions), so
    scores are computed *transposed*: scoresT[k, q] with lhsT = kT (stationary),
    rhs = qT (moving). exp() runs on the ACT engine straight out of PSUM (no
    max subtraction needed: |scores| is O(6) here, exp is safe in fp32).
  - Causality is structural: score tiles with k_block > q are never computed;
    the diagonal 128x128 block gets a 0/1 multiplicative mask after exp.
  - Head pairs (2c, 2c+1) live on partition halves 0-63 / 64-127 of d-chunk c,
    so their K=64 score matmuls land on disjoint PE row groups and overlap.
  - P@V needs no probs transpose: ctxT[d, q] = sum_k v[k, d] * expT[k, q] with
    lhsT = v (natural [s, d] layout), rhs = expT. v is padded to 128 columns
    with 64 replicated ones-columns, so the same matmul emits the softmax
    denominator PRE-BROADCAST on output partitions 64..127 (matmul time is
    set by the moving free dim, so the replicated columns are free). The
    normalization is then just two full-width DVE ops per (head, q-tile):
    reciprocal([64,512] straight out of PSUM) and a multiply — no GPSIMD
    broadcast hop and no 1-partition ops on the critical path.
  - Out-projection contracts d (all heads) from ctxT directly; bv/bo biases
    enter via ones-row K=1 matmuls (bq/bk are per-partition DVE biases).
  - Software pipeline per head-pair iteration: scores(c) [+ P@V(c-1)
    interleaved between score blocks] then the qT/kT projection for chunk
    c+1 — so the ACT exp stream starts early and runs under the PE-heavy
    projection work.
  - Engine balance (default variant "qact+scwide"): q/k projection
    PSUM->SBUF copies run on the ACT engine (the DVE was the critical
    engine once normalization moved there), and exp runs once per
    (head, key-block) over a wide 2-bank PSUM tile to cut ACT instruction
    count.

All shapes/strides hardcoded for this problem.
"""

import os

import numpy as np
import ml_dtypes

import concourse.bass as bass
import concourse.bass_isa as bass_isa
import concourse.mybir as mybir
import concourse.tile as tile
from concourse import bacc
import concourse.bass_utils as bass_utils

B, S, D, H = 8, 1024, 1024, 16
HD = D // H
SCALE = HD ** -0.5
P = 128
NCH = D // P  # 8 chunks of 128
N_CORES = 8

F32 = mybir.dt.float32
BF16 = mybir.dt.bfloat16
EXP = mybir.ActivationFunctionType.Exp
bf16 = ml_dtypes.bfloat16

NEG = -1.0e30


def build_bass(loop_n=None, with_bias=True, variant=None):
    if variant is None:
        variant = os.environ.get("KDIAG", "qact+scwide")
    nc = bacc.Bacc(
        "TRN2",
        target_bir_lowering=False,
        debug=False,
        enable_asserts=False,
        num_devices=N_CORES,
    )

    hsT_d = nc.dram_tensor("hsT", [D, S], BF16, kind="ExternalInput").ap()
    wq_d = nc.dram_tensor("wqT", [D, D], BF16, kind="ExternalInput").ap()
    wk_d = nc.dram_tensor("wkT", [D, D], BF16, kind="ExternalInput").ap()
    wv_d = nc.dram_tensor("wvT", [D, D], BF16, kind="ExternalInput").ap()
    wo_d = nc.dram_tensor("woT", [D, D], BF16, kind="ExternalInput").ap()
    bq_d = nc.dram_tensor("bqc", [P, NCH], F32, kind="ExternalInput").ap()
    bk_d = nc.dram_tensor("bkc", [P, NCH], F32, kind="ExternalInput").ap()
    bv_d = nc.dram_tensor("bvr", [1, D], BF16, kind="ExternalInput").ap()
    bo_d = nc.dram_tensor("bor", [1, D], BF16, kind="ExternalInput").ap()
    mask_d = nc.dram_tensor("maskT", [P, P], BF16, kind="ExternalInput").ap()
    out_d = nc.dram_tensor("out", [S, D], F32, kind="ExternalOutput").ap()

    aps = (hsT_d, wq_d, wk_d, wv_d, wo_d, bq_d, bk_d, bv_d, bo_d, mask_d, out_d)
    with tile.TileContext(nc) as tc:
        pools = (
            tc.alloc_tile_pool(name="persist", bufs=1),
            tc.alloc_tile_pool(name="epool", bufs=2),
            tc.alloc_tile_pool(name="scratch", bufs=2),
            tc.alloc_tile_pool(name="psum", bufs=2, space="PSUM"),
        )
        persist = pools[0]
        # v tile: [v (64 cols) | ones (64 replicated cols)] per (kb, head).
        # The ones half is initialized ONCE outside the iteration loop (it is
        # never overwritten), so it costs nothing in steady state.
        if "ctile" in variant:
            v_sb = persist.tile([P, NCH, H, HD], BF16, name="v_sb")
        else:
            v_sb = persist.tile([P, NCH, H, P], BF16, name="v_sb")
            nc.vector.memset(v_sb[:, :, :, HD:P], 1.0)
        if loop_n is None:
            _kernel_body(tc, pools, v_sb, with_bias, variant, *aps)
        else:
            hints = (
                mybir.EngineType.PE,
                mybir.EngineType.Activation,
                mybir.EngineType.DVE,
                mybir.EngineType.SP,
            )
            with tc.For_i(0, loop_n, 1, hint_engines=hints):
                _kernel_body(tc, pools, v_sb, with_bias, variant, *aps)
        for pool in reversed(pools):
            pool.release()
    nc.compile()
    return nc


def _kernel_body(tc, pools, v_sb, with_bias, variant, hsT_d, wq_d, wk_d, wv_d,
                 wo_d, bq_d, bk_d, bv_d, bo_d, mask_d, out_d):
    nc = tc.nc
    persist, epool, scratch, psum = pools
    use_div = "div" in variant
    psplit = "psplit" in variant
    scwide = "scwide" in variant
    hst2 = "hsT2" in variant
    ctile = "ctile" in variant
    ctx_bufs = 3 if psplit else (4 if scwide else 5)
    e_bufs = 2 if hst2 else 3
    qk_copy_act = "qact" in variant
    mask_eng = nc.gpsimd if "gmask" in variant else nc.vector
    vo_copy = nc.vector.tensor_copy if "vdve" in variant else (
        lambda out, in_: nc.scalar.copy(out, in_))

    # ---- persistent SBUF tensors -------------------------------------------
    if hst2:
        # double-buffered via pool rotation: next iteration's input DMA can
        # land while this iteration still reads the other buffer
        hsT_sb = scratch.tile([P, NCH, S], BF16, tag="hsT", name="hsT_sb", bufs=2)
    else:
        hsT_sb = persist.tile([P, NCH, S], BF16, name="hsT_sb")
    wq_sb = persist.tile([P, NCH, D], BF16, name="wq_sb")
    wk_sb = persist.tile([P, NCH, D], BF16, name="wk_sb")
    wv_sb = persist.tile([P, NCH, D], BF16, name="wv_sb")
    wo_sb = persist.tile([P, NCH, D], BF16, name="wo_sb")
    ctxT_sb = persist.tile([P, NCH, S], BF16, name="ctxT_sb")
    mask_sb = persist.tile([P, P], BF16, name="mask_sb")
    ones_sb = persist.tile([1, P], BF16, name="ones_sb")
    if with_bias:
        bq_sb = persist.tile([P, NCH], F32, name="bq_sb")
        bk_sb = persist.tile([P, NCH], F32, name="bk_sb")
        bv_sb = persist.tile([1, D], BF16, name="bv_sb")
        bo_sb = persist.tile([1, D], BF16, name="bo_sb")

    # ---- input DMAs (per-chunk so compute can start early) ------------------
    hsT_r = hsT_d.rearrange("(c p) s -> c p s", p=P)
    wq_r = wq_d.rearrange("(c p) n -> c p n", p=P)
    wk_r = wk_d.rearrange("(c p) n -> c p n", p=P)
    wv_r = wv_d.rearrange("(c p) n -> c p n", p=P)
    wo_r = wo_d.rearrange("(c p) n -> c p n", p=P)
    # qT proj consumes (hsT[k], wq[k]) pairs in k order — interleave those
    # DMAs so the PE can start ~1us in and stream behind the DMA engines
    for c in range(NCH):
        nc.sync.dma_start(out=hsT_sb[:, c, :], in_=hsT_r[c])
        nc.sync.dma_start(out=wq_sb[:, c, :], in_=wq_r[c])
    for c in range(NCH):
        nc.sync.dma_start(out=wk_sb[:, c, :], in_=wk_r[c])
    for c in range(NCH):
        nc.sync.dma_start(out=wv_sb[:, c, :], in_=wv_r[c])
    for c in range(NCH):
        nc.sync.dma_start(out=wo_sb[:, c, :], in_=wo_r[c])
    if with_bias:
        nc.sync.dma_start(out=bq_sb, in_=bq_d)
        nc.sync.dma_start(out=bk_sb, in_=bk_d)
        nc.sync.dma_start(out=bv_sb, in_=bv_d)
        nc.sync.dma_start(out=bo_sb, in_=bo_d)
    nc.sync.dma_start(out=mask_sb, in_=mask_d)
    nc.vector.memset(ones_sb, 1.0)

    # ---- qT/kT projection for chunk c (heads 2c, 2c+1) ----------------------
    def emit_qk_proj(c):
        outs = []
        for w_sb, bias, nm in (
            (wq_sb, bq_sb[:, c : c + 1] if with_bias else None, "q"),
            (wk_sb, bk_sb[:, c : c + 1] if with_bias else None, "k"),
        ):
            o_sb = scratch.tile([P, S], BF16, tag=f"{nm}T", name=f"{nm}T_{c}", bufs=3)
            for st in range(2):
                ps = psum.tile([P, 512], F32, tag="ctx", name=f"{nm}ps_{c}_{st}",
                               bufs=ctx_bufs)
                for k in range(NCH):
                    nc.tensor.matmul(
                        ps,
                        lhsT=w_sb[:, k, c * P : (c + 1) * P],
                        rhs=hsT_sb[:, k, st * 512 : (st + 1) * 512],
                        start=(k == 0),
                        stop=(k == NCH - 1),
                    )
                dst = o_sb[:, st * 512 : (st + 1) * 512]
                if with_bias:
                    nc.vector.tensor_scalar_add(dst, ps, bias)
                elif qk_copy_act:
                    nc.scalar.copy(dst, ps)
                else:
                    nc.vector.tensor_copy(dst, ps)
            outs.append(o_sb)
        return outs

    # ---- V projection: v[s, d] = hs @ Wv.T + bv  (natural layout) -----------
    def emit_v_pair(m):  # s chunk m, both 512-wide d tiles (8 heads each)
        for nt in range(2):
            ps = psum.tile([P, 512], F32, tag="ctx", name=f"vps_{m}_{nt}",
                           bufs=ctx_bufs)
            for c in range(NCH):
                nc.tensor.matmul(
                    ps,
                    lhsT=hsT_sb[:, c, m * P : (m + 1) * P],
                    rhs=wv_sb[:, c, nt * 512 : (nt + 1) * 512],
                    start=(c == 0),
                    stop=(not with_bias) and (c == NCH - 1),
                )
            if with_bias:
                nc.tensor.matmul(
                    ps,
                    lhsT=ones_sb[:, 0:P],
                    rhs=bv_sb[:, nt * 512 : (nt + 1) * 512],
                    start=False,
                    stop=True,
                )
            vo_copy(
                v_sb[:, m, 8 * nt : 8 * (nt + 1), 0:HD],
                ps.rearrange("p (h e) -> p h e", h=8),
            )

    # ---- scoresT[k, q] + exp for heads 2c, 2c+1 -----------------------------
    # Interleaved so consecutive matmuls sit on disjoint PE row groups
    # (rows 0-63 vs 64-127) and overlap. E[hh][kb] is [128, 1024 - kb*128]
    # covering q in [kb*128, 1024).
    def scores_unit(c, kb, qT, kT, E_pair):
        lo = kb * P
        spans = [(lo, 512), (512, 1024)] if lo < 512 else [(lo, 1024)]
        E_tiles = []
        for hh in range(2):
            h = 2 * c + hh
            E_tiles.append(
                epool.tile([P, S - lo], BF16, tag=f"E{kb}_{hh}",
                           name=f"E_{h}_{kb}", bufs=e_bufs)
            )
            E_pair[hh].append(E_tiles[hh])
        if scwide:
            pssw = [
                psum.tile([P, 1024], F32, tag="sc", name=f"sps_{2*c+hh}_{kb}",
                          bufs=2)
                for hh in range(2)
            ]
            for a, b in spans:
                for hh in range(2):
                    po = hh * 64
                    nc.tensor.matmul(
                        pssw[hh][:, a:b],
                        lhsT=kT[po : po + 64, lo : lo + P],
                        rhs=qT[po : po + 64, a:b],
                        start=True,
                        stop=True,
                    )
            for hh in range(2):
                nc.scalar.activation(E_tiles[hh], pssw[hh][:, lo:1024], EXP)
                mask_eng.tensor_mul(
                    E_tiles[hh][:, 0:P], E_tiles[hh][:, 0:P], mask_sb
                )
            return
        for a, b in spans:
            pss = [
                psum.tile(
                    [P, b - a], F32, tag="sc", name=f"sps_{2*c+hh}_{kb}_{a}", bufs=3
                )
                for hh in range(2)
            ]
            for hh in range(2):
                po = hh * 64
                nc.tensor.matmul(
                    pss[hh],
                    lhsT=kT[po : po + 64, lo : lo + P],
                    rhs=qT[po : po + 64, a:b],
                    start=True,
                    stop=True,
                )
            for hh in range(2):
                nc.scalar.activation(E_tiles[hh][:, a - lo : b - lo], pss[hh], EXP)
                if a == lo:
                    # exp(s + mask) == exp(s) * (mask == 0): zero the upper
                    # triangle of the diagonal block (bf16 2x DVE mode)
                    mask_eng.tensor_mul(
                        E_tiles[hh][:, 0:P], E_tiles[hh][:, 0:P], mask_sb
                    )

    # ---- P@V + normalization for one (head, q-tile) -------------------------
    # M=128 matmul: output rows 0-63 = ctx, rows 64-127 = denominator
    # replicated across 64 partitions by the 64 ones-columns of v.
    def pv_unit(c, hh, qt, E_pair):
        h = 2 * c + hh
        po = hh * 64
        E = E_pair[hh]
        if psplit:
            cps = psum.tile([P, 512], F32, tag="pvp", name=f"cps_{h}_{qt}", bufs=2)
        else:
            cps = psum.tile([P, 512], F32, tag="ctx", name=f"cps_{h}_{qt}",
                            bufs=ctx_bufs)
        kmax = 3 if qt == 0 else 7
        for kb in range(kmax + 1):
            lo = kb * P
            off = max(0, lo - qt * 512)
            nc.tensor.matmul(
                cps[:, off:512],
                lhsT=v_sb[:, kb, h, :],
                rhs=E[kb][:, qt * 512 + off - lo : (qt + 1) * 512 - lo],
                start=(kb == 0),
                stop=(kb == kmax),
            )
        if use_div:
            # fused: ctx = num / den in one DVE op (den pre-broadcast on
            # psum partitions 64-127 by the replicated ones-columns of v)
            nc.vector.tensor_tensor(
                ctxT_sb[po : po + 64, c, qt * 512 : (qt + 1) * 512],
                cps[0:HD, :],
                cps[64:128, :],
                mybir.AluOpType.divide,
            )
            return
        rec = scratch.tile([64, 512], F32, tag="rec", name=f"rec_{h}_{qt}", bufs=3)
        nc.vector.reciprocal(rec, cps[64:128, :])
        nc.vector.tensor_mul(
            ctxT_sb[po : po + 64, c, qt * 512 : (qt + 1) * 512],
            cps[0:HD, :],
            rec,
        )

    # ---- ctile: col-group-tiled P@V pair + off-PE denominators -------------
    def pv_pair_unit(c, qt, E_pair, den_bc):
        cps = psum.tile([P, 512], F32, tag="ctx", name=f"cpp_{c}_{qt}",
                        bufs=ctx_bufs)
        kmax = 3 if qt == 0 else 7
        for kb in range(kmax + 1):
            lo = kb * P
            off = max(0, lo - qt * 512)
            for hh in range(2):
                nc.tensor.matmul(
                    cps[64 * hh : 64 * hh + 64, off:512],
                    lhsT=v_sb[:, kb, 2 * c + hh, 0:HD],
                    rhs=E_pair[hh][kb][:, qt * 512 + off - lo : (qt + 1) * 512 - lo],
                    start=(kb == 0),
                    stop=(kb == kmax),
                    tile_position=(0, 64 * hh),
                )
        for hh in range(2):
            h = 2 * c + hh
            rec = scratch.tile([64, 512], F32, tag="rec", name=f"rcc_{h}_{qt}",
                               bufs=3)
            nc.vector.reciprocal(
                rec, den_bc[0:64, hh, qt * 512 : (qt + 1) * 512]
            )
            nc.vector.tensor_mul(
                ctxT_sb[64 * hh : 64 * hh + 64, c, qt * 512 : (qt + 1) * 512],
                cps[64 * hh : 64 * hh + 64, :],
                rec,
            )

    def emit_fold(c, E_pair, den_acc, den_bc):
        # denominator fold, emitted AFTER the next chunk's projection so the
        # DVE-critical copies are not blocked behind exp-dependent adds
        for hh in range(2):
            nc.vector.tensor_copy(den_acc[:, hh, :], E_pair[hh][0])
            for kb in range(1, NCH):
                lo = kb * P
                nc.vector.tensor_add(
                    den_acc[:, hh, lo:S], den_acc[:, hh, lo:S], E_pair[hh][kb]
                )
            nc.gpsimd.partition_all_reduce(
                den_bc[:, hh, :], den_acc[:, hh, :], channels=P,
                reduce_op=bass_isa.ReduceOp.add,
            )

    # ---- pipeline ----------------------------------------------------------
    qk = emit_qk_proj(0)
    prev_E = None
    prev_den_bc = None
    F16 = mybir.dt.float16
    for c in range(NCH):
        cur_E = [[], []]
        if ctile:
            den_acc = scratch.tile([P, 2, S], F16, tag="den", name=f"den_{c}",
                                   bufs=2)
            den_bc = scratch.tile([P, 2, S], F16, tag="denb", name=f"denb_{c}",
                                  bufs=2)
        for kb in range(NCH):
            scores_unit(c, kb, qk[0], qk[1], cur_E)
            if c == 0:
                # weave the V projection into iteration 0 (its pv slot is
                # empty) so its DMA waits hide under score matmuls
                emit_v_pair(kb)
            elif ctile:
                if kb == 3:
                    pv_pair_unit(c - 1, 0, prev_E, prev_den_bc)
                elif kb == 7:
                    pv_pair_unit(c - 1, 1, prev_E, prev_den_bc)
            elif kb % 2 == 1:
                # weave previous pair's P@V between score blocks so the PE
                # has ready work while exp() drains score PSUM slots
                g = kb // 2
                pv_unit(c - 1, g // 2, g % 2, prev_E)
        if c + 1 < NCH:
            qk = emit_qk_proj(c + 1)
        if ctile:
            emit_fold(c, cur_E, den_acc, den_bc)
            prev_den_bc = den_bc
        prev_E = cur_E
    if ctile:
        for qt in range(2):
            pv_pair_unit(NCH - 1, qt, prev_E, prev_den_bc)
    else:
        for g in range(4):
            pv_unit(NCH - 1, g // 2, g % 2, prev_E)

    # ---- out projection: out[q, d_o] = ctx @ Wo.T + bo ----------------------
    for qb in range(NCH):
        for nt in range(2):
            ps = psum.tile([P, 512], F32, tag="ctx", name=f"ops_{qb}_{nt}",
                           bufs=ctx_bufs)
            for c in range(NCH):
                nc.tensor.matmul(
                    ps,
                    lhsT=ctxT_sb[:, c, qb * P : (qb + 1) * P],
                    rhs=wo_sb[:, c, nt * 512 : (nt + 1) * 512],
                    start=(c == 0),
                    stop=(not with_bias) and (c == NCH - 1),
                )
            if with_bias:
                nc.tensor.matmul(
                    ps,
                    lhsT=ones_sb[:, 0:P],
                    rhs=bo_sb[:, nt * 512 : (nt + 1) * 512],
                    start=False,
                    stop=True,
                )
            osb = scratch.tile([P, 512], F32, tag="osb", name=f"osb_{qb}_{nt}", bufs=3)
            vo_copy(osb, ps)
            nc.sync.dma_start(
                out=out_d[qb * P : (qb + 1) * P, nt * 512 : (nt + 1) * 512], in_=osb
            )


def make_in_maps(hidden_states, causal_attention_mask, Wq, bq, Wk, bk, Wv, bv, Wo, bo):
    f32 = np.float32
    wqT = np.ascontiguousarray((np.asarray(Wq, f32) * SCALE).T).astype(bf16)
    wkT = np.ascontiguousarray(np.asarray(Wk, f32).T).astype(bf16)
    wvT = np.ascontiguousarray(np.asarray(Wv, f32).T).astype(bf16)
    woT = np.ascontiguousarray(np.asarray(Wo, f32).T).astype(bf16)
    bqc = np.ascontiguousarray((np.asarray(bq, f32) * SCALE).reshape(NCH, P).T)
    bkc = np.ascontiguousarray(np.asarray(bk, f32).reshape(NCH, P).T)
    bvr = np.asarray(bv, f32).reshape(1, D).astype(bf16)
    bor = np.asarray(bo, f32).reshape(1, D).astype(bf16)
    # diagonal-block mask, transposed to [k, q], as a 0/1 multiplicative mask
    # (exp(s + m) == exp(s) * [m == 0] for the causal 0/-inf mask)
    mblk = np.asarray(causal_attention_mask, f32)[0, 0, :P, :P]
    maskT = (np.ascontiguousarray(mblk.T) >= 0).astype(bf16)
    shared = {
        "wqT": wqT, "wkT": wkT, "wvT": wvT, "woT": woT,
        "bqc": bqc, "bkc": bkc, "bvr": bvr, "bor": bor, "maskT": maskT,
    }
    hs = np.asarray(hidden_states, f32)
    in_maps = []
    for b in range(B):
        m = dict(shared)
        m["hsT"] = np.ascontiguousarray(hs[b].astype(bf16).T)
        in_maps.append(m)
    return in_maps


_NC_CACHE = {}


def get_nc(with_bias=True):
    if with_bias not in _NC_CACHE:
        _NC_CACHE[with_bias] = build_bass(with_bias=with_bias)
    return _NC_CACHE[with_bias]


def kernel(hidden_states, causal_attention_mask, Wq, bq, Wk, bk, Wv, bv, Wo, bo,
           **run_kwargs):
    with_bias = not (
        np.all(np.asarray(bq) == 0)
        and np.all(np.asarray(bk) == 0)
        and np.all(np.asarray(bv) == 0)
        and np.all(np.asarray(bo) == 0)
    )
    nc = get_nc(with_bias=with_bias)
    in_maps = make_in_maps(
        hidden_states, causal_attention_mask, Wq, bq, Wk, bk, Wv, bv, Wo, bo
    )
    res = bass_utils.run_bass_kernel_spmd(
        nc, in_maps, core_ids=list(range(N_CORES)), **run_kwargs
    )
    out = np.stack([res.results[i]["out"] for i in range(N_CORES)])
    kernel.last_results = res
    return out

